# revision 13
# baseline (speedup 1.0000x reference)
"""Fused GEMM + bias + residual + AvgPool2d(2) + global-mean normalize, 8-core SPMD.

Reference computation (B=8192, IN_F=1024, OUT_F=4096, S=64, K=2):
    out_lin = x @ W.T + bias + y                  # (B, 4096)
    pooled  = avgpool2x2(out_lin.reshape(B,64,64))# (B, 32, 32)
    out     = pooled / pooled.mean()              # (B, 1, 32, 32)

Algebraic folds (exact):
  * The 2x2 avg-pool is linear -> folds into weight/bias/residual:
        pooled_raw[b, m] = x[b] . Wsum[m] + bias_sum[m] + y_sum[b, m]
    with m = 32*i + j pooling OUT_F rows {128i+64r+2j+s : r,s in {0,1}}.
    GEMM N-dim shrinks 4096 -> 1024; the (B, 4096) intermediate is never
    materialized.
  * The 1/4 pool factor cancels against the global mean:
        out = pooled_raw * (B*1024 / sum_global(pooled_raw))
  * The global sum decomposes over raw inputs:
        local_sum = xsum . wcolsum + BL * bias_tot + ytot

Host staging (sharding/layout only): inputs are cast to bf16 and laid out
in SBUF-tile-major order on the host -- x and W pre-transposed to [K, *],
and the 4096-wide pooled axes of W and y permuted to (q=rs, m) order.
The 4-tap pooling then runs almost entirely inside the DMA datapath:
SWDGE accumulate-DMAs (CCE add) fold tap pairs q0+q1 / q2+q3 while
loading, and one DVE add per pair finishes the sum.  Per-core HBM
traffic is 18 MiB in + 4 MiB out and the kernel has no on-chip
transposes.

Cross-core scalar sum: either one collective AllReduce (USE_REMOTE=False)
or a low-latency XOR all-to-all built from 7 single-destination
remote_dma_broadcast calls (USE_REMOTE=True): slot d of the receive
buffer gets the partial sum of core (me XOR d), so one compile-time SPMD
program needs no core ids.

Sharding: batch B split 8 ways (1024 rows/core); weight + bias replicated.
"""

import numpy as np
import ml_dtypes

import concourse.bass as bass
import concourse.mybir as mybir
import concourse.tile as tile
from concourse import bacc
from concourse.bass import ts
from concourse.bass_utils import run_bass_kernel_spmd

N_CORES = 8
B = 8192
BL = B // N_CORES          # 1024 batch rows per core
KF = 1024                  # IN_F (contraction)
NF = 4096                  # OUT_F
M = 1024                   # pooled features (32*32)
TOT = float(B * M)         # elements in the global mean
F32 = mybir.dt.float32
BF16 = mybir.dt.bfloat16
ADD = mybir.AluOpType.add
MULT = mybir.AluOpType.mult

USE_REMOTE = False          # scalar exchange: remote_dma vs collective

_CACHE = {}


def _pool_perm():
    """n-axis permutation: q=(r,s)-major, pooled-feature-minor order."""
    m = np.arange(M)
    i, j = m // 32, m % 32
    cols = []
    for r in (0, 1):
        for s in (0, 1):
            cols.append(128 * i + 64 * r + 2 * j + s)
    return np.concatenate(cols)


def build_nc(use_remote=USE_REMOTE):
    nc = bacc.Bacc("TRN2", target_bir_lowering=False, debug=False,
                   num_devices=N_CORES)
    # host-staged layouts (bf16): xt [k-part, kt, b]; wt/y [part, q, t, m]
    xt = nc.dram_tensor("xt", [128, 8, BL], BF16, kind="ExternalInput").ap()
    wt = nc.dram_tensor("wt", [128, 4, 8, M], BF16,
                        kind="ExternalInput").ap()
    yd = nc.dram_tensor("y", [128, 4, 8, M], BF16, kind="ExternalInput").ap()
    bd = nc.dram_tensor("b", [1, NF], BF16, kind="ExternalInput").ap()
    out = nc.dram_tensor("out", [128, 8, M], F32, kind="ExternalOutput").ap()

    ring = [nc.sync, nc.scalar]
    BYP = mybir.AluOpType.bypass

    with tile.TileContext(nc) as tc:
        with (
            tc.tile_pool(name="consts", bufs=1) as consts,
            tc.tile_pool(name="xtp", bufs=1) as xtp,
            tc.tile_pool(name="pairp", bufs=1) as pairp,
            tc.tile_pool(name="wsp", bufs=1) as wsp,
            tc.tile_pool(name="ysp", bufs=1) as ysp,
            tc.tile_pool(name="orp", bufs=1) as orp,
            tc.tile_pool(name="statsp", bufs=1) as statsp,
            tc.tile_pool(name="outp", bufs=4) as outp,
            tc.tile_pool(name="psA", bufs=4, space="PSUM") as psA,
            tc.tile_pool(name="psB", bufs=4, space="PSUM") as psB,
            tc.tile_pool(name="dram", bufs=1, space="DRAM") as dram,
        ):
            # ---- constants ----
            ones_row_bf = consts.tile([1, 128], BF16)
            nc.vector.memset(ones_row_bf, 1.0)
            ones_row_f = consts.tile([1, 128], F32)
            nc.vector.memset(ones_row_f, 1.0)
            ones_col = consts.tile([128, 1], F32)
            nc.vector.memset(ones_col, 1.0)
            ones_one = consts.tile([1, 1], F32)
            nc.vector.memset(ones_one, 1.0)

            # ---- bias: load, pool 4096 -> 1024 (raw order), totals ----
            bload = consts.tile([1, NF], BF16)
            nc.sync.dma_start(out=bload, in_=bd)
            blv = bload.rearrange("o (i r j s) -> o i r j s", r=2, j=32, s=2)
            bsum = consts.tile([1, 32, 32], F32)
            nc.vector.tensor_add(bsum, blv[:, :, 0, :, 0], blv[:, :, 0, :, 1])
            nc.vector.tensor_add(bsum, bsum, blv[:, :, 1, :, 0])
            nc.vector.tensor_add(bsum, bsum, blv[:, :, 1, :, 1])
            bsum_bf = consts.tile([1, M], BF16)
            nc.vector.tensor_copy(out=bsum_bf,
                                  in_=bsum.rearrange("o i j -> o (i j)"))
            btot = consts.tile([1, 1], F32)
            nc.vector.reduce_sum(out=btot,
                                 in_=bsum.rearrange("o i j -> o (i j)"),
                                 axis=mybir.AxisListType.X)
            btot_s = consts.tile([1, 1], F32)
            nc.scalar.mul(btot_s, btot, float(BL))

            # ---- x^T on HWDGE rings (first), resident ----
            xts = xtp.tile([128, 8, BL], BF16)
            ring[0].dma_start(out=xts[:, 0:4, :], in_=xt[:, 0:4, :])
            ring[1].dma_start(out=xts[:, 4:8, :], in_=xt[:, 4:8, :])

            # ---- W/y arrive as: big HWDGE bypass-writes of taps q0/q2
            # (full-rate, two rings), SWDGE accumulate-DMAs of taps q1/q3
            # into disjoint slices (each <= 2048 elems/partition, the CCE
            # limit), then one DVE add per pair of tap-sums. ----
            wAall = pairp.tile([128, 8, M], BF16, tag="WA", name="wAall")
            wBall = pairp.tile([128, 8, M], BF16, tag="WB", name="wBall")
            yAall = pairp.tile([128, 8, M], BF16, tag="YA", name="yAall")
            yBall = pairp.tile([128, 8, M], BF16, tag="YB", name="yBall")
            ring[0].dma_start(out=wAall, in_=wt[:, 0, :, :])
            ring[1].dma_start(out=wBall, in_=wt[:, 2, :, :])
            ring[0].dma_start(out=yAall, in_=yd[:, 0, :, :])
            ring[1].dma_start(out=yBall, in_=yd[:, 2, :, :])

            wsum_all = wsp.tile([128, 8, M], BF16)
            for p in range(4):
                nc.gpsimd.dma_start(out=wAall[:, ts(p, 2), :],
                                    in_=wt[:, 1, ts(p, 2), :], accum_op=ADD)
                nc.gpsimd.dma_start(out=wBall[:, ts(p, 2), :],
                                    in_=wt[:, 3, ts(p, 2), :], accum_op=ADD)
                nc.vector.tensor_add(wsum_all[:, ts(p, 2), :],
                                     wAall[:, ts(p, 2), :],
                                     wBall[:, ts(p, 2), :])

            # ---- y accumulates + per-group finish ----
            ys_all = ysp.tile([128, 8, M], BF16)
            yt = statsp.tile([128, 8, 1], F32)

            def yfinish(g):
                nc.vector.tensor_add(ys_all[:, ts(g, 2), :],
                                     yAall[:, ts(g, 2), :],
                                     yBall[:, ts(g, 2), :])
                nc.vector.reduce_sum(out=yt[:, ts(g, 2), :],
                                     in_=ys_all[:, ts(g, 2), :],
                                     axis=mybir.AxisListType.X)

            for g in range(4):
                nc.gpsimd.dma_start(out=yAall[:, ts(g, 2), :],
                                    in_=yd[:, 1, ts(g, 2), :], accum_op=ADD)
                nc.gpsimd.dma_start(out=yBall[:, ts(g, 2), :],
                                    in_=yd[:, 3, ts(g, 2), :], accum_op=ADD)

            # ---- inputs-side stats whose data is ready early ----
            xs = statsp.tile([128, 8, 1], F32)
            nc.vector.reduce_sum(out=xs, in_=xts, axis=mybir.AxisListType.X)
            wcol = statsp.tile([128, 8, 1], F32)
            nc.vector.reduce_sum(out=wcol, in_=wsum_all,
                                 axis=mybir.AxisListType.X)
            combo = statsp.tile([128, 16], F32)
            nc.vector.tensor_mul(combo[:, 0:8], xs[:, :, 0], wcol[:, :, 0])

            # ---- GEMM in two 4-bt groups, kb-major (keeps PE ramped);
            # per-engine emission follows expected data-arrival order so
            # no engine head-of-line blocks on late data ----
            or_tiles = {}

            def gemm_matmuls(bts):
                mm = {}
                for bt in bts:
                    mm[bt] = [psA.tile([128, 512], F32, tag="mmA",
                                       name=f"mmA{bt}"),
                              psB.tile([128, 512], F32, tag="mmB",
                                       name=f"mmB{bt}")]
                for kb in range(8):
                    for bt in bts:
                        for mh in range(2):
                            nc.tensor.matmul(mm[bt][mh],
                                             xts[:, kb, ts(bt, 128)],
                                             wsum_all[:, kb, ts(mh, 512)],
                                             start=(kb == 0), stop=False)
                return mm

            def gemm_close(mm, bts):
                for bt in bts:
                    orb = orp.tile([128, M], BF16, tag=f"or{bt}",
                                   name=f"or{bt}")
                    or_tiles[bt] = orb
                    for mh in range(2):
                        nc.tensor.matmul(mm[bt][mh], ones_row_bf,
                                         bsum_bf[:, ts(mh, 512)],
                                         start=False, stop=True)
                        nc.vector.tensor_add(orb[:, ts(mh, 512)], mm[bt][mh],
                                             ys_all[:, bt, ts(mh, 512)])

            mmA_ = gemm_matmuls([0, 1, 2, 3])
            yfinish(0)
            yfinish(1)
            gemm_close(mmA_, [0, 1, 2, 3])
            yfinish(2)
            yfinish(3)
            mmB_ = gemm_matmuls([4, 5, 6, 7])

            nc.vector.tensor_copy(out=combo[:, 8:16], in_=yt[:, :, 0])
            part = statsp.tile([128, 1], F32)
            nc.vector.reduce_sum(out=part, in_=combo,
                                 axis=mybir.AxisListType.X)

            gemm_close(mmB_, [4, 5, 6, 7])

            # ---- local sum -> global sum exchange ----
            ls_ps = psA.tile([1, 1], F32, tag="mmA", name="ls_ps")
            nc.tensor.matmul(ls_ps, part, ones_col, start=True, stop=False)
            nc.tensor.matmul(ls_ps, btot_s, ones_one, start=False, stop=True)
            ls2 = statsp.tile([1, 1], F32)
            nc.scalar.copy(out=ls2, in_=ls_ps)

            rsb = statsp.tile([128, 1], F32)
            if use_remote:
                bc_ps = psB.tile([128, 1], F32, tag="mmB", name="bc_ps")
                nc.tensor.matmul(bc_ps, ones_row_f, ls2, start=True,
                                 stop=True)
                loc128 = statsp.tile([128, 1], F32)
                nc.scalar.copy(out=loc128, in_=bc_ps)

                slots = statsp.tile([128, 8], F32)
                rsem = nc.alloc_semaphore("xch_recv")
                lsem = nc.alloc_semaphore("xch_sent")
                nc.vector.tensor_copy(out=slots[:, 0:1], in_=loc128)
                for d in range(1, 8):
                    rdests = [None] * 8
                    rdests[d] = (0, d)
                    nc.gpsimd.remote_dma_broadcast(
                        out_ap=slots[:, d:d + 1], in_ap=loc128,
                        remote_sem=rsem, local_sem=lsem, rdests=rdests)
                nc.gpsimd.trigger_dma(count=None)
                nc.vector.wait_ge(rsem, 14)
                gs = statsp.tile([128, 1], F32)
                nc.vector.reduce_sum(out=gs, in_=slots,
                                     axis=mybir.AxisListType.X)
                nc.vector.reciprocal(rsb, gs)
                nc.gpsimd.wait_ge(lsem, 112)
                nc.clear_and_free_semaphores([rsem, lsem])
            else:
                cc_in = dram.tile([1, 1], F32)
                cc_out = dram.tile([1, 1], F32)
                nc.sync.dma_start(out=cc_in, in_=ls2)
                nc.gpsimd.collective_compute(
                    "AllReduce", ADD,
                    replica_groups=[list(range(N_CORES))],
                    ins=[cc_in.opt()], outs=[cc_out.opt()])
                gsb = statsp.tile([128, 1], F32)
                nc.sync.dma_start(out=gsb, in_=cc_out.to_broadcast((128, 1)))
                nc.vector.reciprocal(rsb, gsb)

            # ---- normalize + store: out = pooled * (1/gsum) * TOT ----
            for bt in range(8):
                meng = nc.vector if bt % 2 == 0 else nc.gpsimd
                ot = outp.tile([128, M], F32)
                meng.tensor_scalar(out=ot, in0=or_tiles[bt],
                                   scalar1=rsb, scalar2=TOT,
                                   op0=MULT, op1=MULT)
                ring[bt % 2].dma_start(out=out[:, bt, :], in_=ot)

    nc.compile()
    return nc


def _stage_x(a):
    """x slice [BL, KF] -> transpose -> bf16 [128, 8, BL]."""
    t = a.T.astype(ml_dtypes.bfloat16).reshape(8, 128, BL)
    return np.ascontiguousarray(t.transpose(1, 0, 2))


def _stage_pooled(a, perm, trans):
    """[rows, 4096] (optionally via transpose) -> bf16 [128, 4, T, M]."""
    if trans:
        a = a.T
    a = a[:, perm]                                   # (q, m) order
    r = a.shape[0]
    t = a.astype(ml_dtypes.bfloat16).reshape(r // 128, 128, 4, M)
    return np.ascontiguousarray(t.transpose(1, 2, 0, 3))


def _run(inputs, trace=False):
    if "nc" not in _CACHE:
        _CACHE["nc"] = build_nc()
    nc = _CACHE["nc"]
    x = np.asarray(inputs["x"], dtype=np.float32)
    y = np.asarray(inputs["y"], dtype=np.float32)
    w = np.asarray(inputs["weight"], dtype=np.float32)
    b = np.asarray(inputs["bias"], dtype=np.float32).reshape(1, NF)
    b = b.astype(ml_dtypes.bfloat16)
    perm = _pool_perm()
    wt_host = _stage_pooled(w, perm, trans=True)     # [128, 4, 8, 1024]
    in_maps = [
        {"xt": _stage_x(x[c * BL:(c + 1) * BL]),
         "y": _stage_pooled(y[c * BL:(c + 1) * BL], perm, trans=False),
         "wt": wt_host, "b": b}
        for c in range(N_CORES)
    ]
    res = run_bass_kernel_spmd(nc, in_maps, core_ids=list(range(N_CORES)),
                               trace=trace)
    full = np.concatenate(
        [res.results[c]["out"].transpose(1, 0, 2).reshape(BL, M)
         for c in range(N_CORES)], axis=0)
    return full.astype(np.float32).reshape(B, 1, 32, 32), res


def kernel(**inputs) -> np.ndarray:
    out, _ = _run(inputs, trace=False)
    return out
